# revision 16
# baseline (speedup 1.0000x reference)
"""Trainium2 Bass kernel for nn_Attention_68006512164916.

EVA-style vision attention block: qkv proj -> 2D rope (interleaved pairs)
-> SDPA (16 heads, d=64, seq 256) -> out proj. B=64, N=256, C=1024, fp32 I/O.

Strategy: data-parallel over batch across 8 NeuronCores (8 batches/core,
no collectives). Per core, everything is computed in bf16 on the
TensorEngine with fp32 PSUM accumulation:

  - host: x is transposed/cast to xT [C, B_loc*N] bf16 so the QKV matmul
    needs no on-device transpose of x; qkv_w rows for q/k are permuted
    (per-head d-interleave -> [evens|odds]) so rope becomes half-block
    free-axis ops; q rows pre-scaled by D^-0.5; proj_w pre-transposed.
  - qkv = xT.T @ wT  (option A layout [n, o]) -> PSUM -> bf16 SBUF
  - rope applied as 6 DVE tensor_tensor ops per [128,1024] tile
  - q,k transposed per 128x128 block on the TensorEngine (identity mm)
    into qT/kT [d, n] layout for attention
  - S^T[j,i] = kT.T @ qT (K=64), exp on ScalarE (no max subtraction:
    |scores| <= ~6 for this input distribution), row sums over j via a
    one-hot-selector matmul into a [16, 256] PSUM tile, 1/sum on DVE,
    broadcast via a DRAM bounce, O^T = v.T-free PV matmul, normalize
    fused into the PSUM eviction.
  - y = O^T.T @ pwT + b, streamed out as fp32.
"""

import sys

if "/opt/trn_rl_repo" not in sys.path:
    sys.path.insert(0, "/opt/trn_rl_repo")

import numpy as np
import ml_dtypes

import concourse.bacc as bacc
import concourse.mybir as mybir
import concourse.tile as tile
from concourse.bass_utils import run_bass_kernel_spmd
from concourse.masks import make_identity

f32 = mybir.dt.float32
bf16 = mybir.dt.bfloat16

N_CORES = 8
B, N, C = 64, 256, 1024
H, D = 16, 64
B_LOC = B // N_CORES          # 8 batches per core
NT = B_LOC * N                # 2048 token rows per core
HW = 16
THETA = 10000.0

_cache = {}


def _rope_tables():
    hd = D // 2  # 32
    inv_freq = 1.0 / (THETA ** (np.arange(0, hd, 2, dtype=np.float32) / hd))
    t = np.arange(HW, dtype=np.float32)
    f = np.einsum("i,j->ij", t, inv_freq)          # (16, 16)
    f = np.repeat(f, 2, axis=-1)                   # (16, 32)
    fx = np.broadcast_to(f[:, None, :], (HW, HW, hd))
    fy = np.broadcast_to(f[None, :, :], (HW, HW, hd))
    F = np.concatenate([fx, fy], axis=-1).reshape(N, D)  # (256, 64)
    cosH = np.cos(F[:, 0::2])                      # (256, 32)
    sinH = np.sin(F[:, 0::2])
    return cosH.astype(np.float32), sinH.astype(np.float32)


def _build():
    if "nc" in _cache:
        return _cache["nc"]

    nc = bacc.Bacc("TRN2", target_bir_lowering=False, debug=False,
                   num_devices=N_CORES)

    xT_d = nc.dram_tensor("xT", [C, NT], bf16, kind="ExternalInput")
    w_d = nc.dram_tensor("wT", [C, 3 * C], bf16, kind="ExternalInput")
    pw_d = nc.dram_tensor("pwT", [C, C], bf16, kind="ExternalInput")
    pb_d = nc.dram_tensor("pb", [1, C], f32, kind="ExternalInput")
    cos_d = nc.dram_tensor("cosH", [N, 32], bf16, kind="ExternalInput")
    sin_d = nc.dram_tensor("sinH", [N, 32], bf16, kind="ExternalInput")
    out_d = nc.dram_tensor("out", [NT, C], f32, kind="ExternalOutput")

    Exp = mybir.ActivationFunctionType.Exp
    MUL = mybir.AluOpType.mult
    ADD = mybir.AluOpType.add
    SUB = mybir.AluOpType.subtract

    from contextlib import ExitStack
    with tile.TileContext(nc) as tc:
        with ExitStack() as ctx:
            const = ctx.enter_context(tc.tile_pool(name="const", bufs=1))
            xg_p = ctx.enter_context(tc.tile_pool(name="xg", bufs=2))
            qkraw_p = ctx.enter_context(tc.tile_pool(name="qkraw", bufs=2))
            tmp_p = ctx.enter_context(tc.tile_pool(name="tmp", bufs=4))
            qkrot_p = ctx.enter_context(tc.tile_pool(name="qkrot", bufs=9))
            v_p = ctx.enter_context(tc.tile_pool(name="vg", bufs=2))
            qkT_p = ctx.enter_context(tc.tile_pool(name="qkT", bufs=2))
            pT_p = ctx.enter_context(tc.tile_pool(name="pT", bufs=18))
            oT_p = ctx.enter_context(tc.tile_pool(name="oT", bufs=2))
            recip_p = ctx.enter_context(tc.tile_pool(name="recip", bufs=3))
            recipbc_p = ctx.enter_context(tc.tile_pool(name="recipbc", bufs=3))
            y_p = ctx.enter_context(tc.tile_pool(name="y", bufs=2))
            dram_p = ctx.enter_context(tc.tile_pool(name="dram", bufs=3, space="DRAM"))
            psmm_p = ctx.enter_context(tc.tile_pool(name="psmm", bufs=4, space="PSUM"))
            pso_p = ctx.enter_context(tc.tile_pool(name="pso", bufs=2, space="PSUM"))
            pssum_p = ctx.enter_context(tc.tile_pool(name="pssum", bufs=2, space="PSUM"))

            # ---- constants ----
            # split the 6MB weight load into per-chunk DMAs spread across
            # the per-engine DMA queues so they run in parallel and the
            # first qkv matmuls can start as soon as chunk 0 lands
            dma_engines = [nc.sync, nc.scalar, nc.gpsimd]
            w_sb = const.tile([128, 8, 3 * C], bf16)
            w_r = w_d.ap().rearrange("(co ci) o -> ci co o", ci=128)
            for cc in range(8):
                dma_engines[cc % 3].dma_start(
                    w_sb[:, cc:cc + 1, :], w_r[:, cc:cc + 1, :])
            cos_sb = const.tile([128, 2, 32], bf16)
            nc.sync.dma_start(cos_sb[:], cos_d.ap().rearrange(
                "(nt p) t -> p nt t", p=128))
            sin_sb = const.tile([128, 2, 32], bf16)
            nc.sync.dma_start(sin_sb[:], sin_d.ap().rearrange(
                "(nt p) t -> p nt t", p=128))
            # selector for head sums: selc[:, 15] == 1; E_r = selc[:, 15-r:23-r]
            selc = const.tile([128, 31], bf16)
            nc.vector.memset(selc[:], 0.0)
            nc.vector.memset(selc[:, 15:16], 1.0)
            # loads not needed until the first projection (~100us in)
            pw_sb = const.tile([128, 8, C], bf16)
            nc.sync.dma_start(pw_sb[:], pw_d.ap().rearrange(
                "(co ci) o -> ci co o", ci=128))
            pb_bc = const.tile([128, C], f32)
            nc.sync.dma_start(pb_bc[:], pb_d.ap().to_broadcast((128, C)))

            xT_r = xT_d.ap().rearrange("(co ci) n -> ci co n", ci=128)

            for g in range(4):          # group = 2 batches (512 token cols)
                xg = xg_p.tile([128, 8, 512], bf16, tag="xg")
                if g == 0:
                    # split the first x load too: the first matmul only
                    # needs chunk 0, so don't serialize behind 1MB
                    for cc in range(8):
                        dma_engines[(cc + 1) % 3].dma_start(
                            xg[:, cc:cc + 1, :],
                            xT_r[:, cc:cc + 1, g * 512:(g + 1) * 512])
                else:
                    nc.sync.dma_start(xg[:], xT_r[:, :, g * 512:(g + 1) * 512])
                v_g = v_p.tile([128, 4, C], bf16, tag="vg")
                qkT_g = qkT_p.tile([128, 16, 512], bf16, tag="qkT")
                rot_tiles = {}

                # ---- qkv matmuls + rope ----
                # the two 512-wide halves of each 1024-col output share the
                # same stationary lhsT per k-chunk, letting walrus reuse the
                # loaded weights between consecutive matmuls
                for fp in range(3):     # 0: q, 1: k, 2: v (1024 cols each)
                    for ns in range(4):
                        if fp < 2:
                            raw = qkraw_p.tile([128, H, D], bf16, tag="qkraw")
                            rawf = raw[:].rearrange("p h d -> p (h d)")
                        pss = [psmm_p.tile([128, 512], f32, tag="mm",
                                           name=f"ps_{fp}_{ns}_{i}")
                               for i in range(2)]
                        for cc in range(8):
                            for half in range(2):
                                fo = fp * 2 + half
                                nc.tensor.matmul(
                                    pss[half][:],
                                    lhsT=xg[:, cc, ns * 128:(ns + 1) * 128],
                                    rhs=w_sb[:, cc, fo * 512:(fo + 1) * 512],
                                    start=(cc == 0), stop=(cc == 7))
                        for half in range(2):
                            if fp == 2:
                                nc.any.tensor_copy(
                                    out=v_g[:, ns, half * 512:(half + 1) * 512],
                                    in_=pss[half][:])
                            else:
                                nc.any.tensor_copy(
                                    out=rawf[:, half * 512:(half + 1) * 512],
                                    in_=pss[half][:])
                        if fp == 2:
                            continue
                        # rope: evens = raw[:,:,0:32], odds = raw[:,:,32:64]
                        nt = ns % 2
                        cos = cos_sb[:, nt, None, :].to_broadcast((128, H, 32))
                        sin = sin_sb[:, nt, None, :].to_broadcast((128, H, 32))
                        qe = raw[:, :, 0:32]
                        qo = raw[:, :, 32:64]
                        t1 = tmp_p.tile([128, H, 32], bf16, tag="tmp")
                        t2 = tmp_p.tile([128, H, 32], bf16, tag="tmp")
                        nc.vector.tensor_tensor(out=t1[:], in0=qe, in1=cos, op=MUL)
                        nc.vector.tensor_tensor(out=t2[:], in0=qo, in1=sin, op=MUL)
                        rot = qkrot_p.tile([128, H, D], bf16, tag="rot")
                        nc.vector.tensor_tensor(out=rot[:, :, 0:32],
                                                in0=t1[:], in1=t2[:], op=SUB)
                        t3 = tmp_p.tile([128, H, 32], bf16, tag="tmp")
                        t4 = tmp_p.tile([128, H, 32], bf16, tag="tmp")
                        nc.vector.tensor_tensor(out=t3[:], in0=qo, in1=cos, op=MUL)
                        nc.vector.tensor_tensor(out=t4[:], in0=qe, in1=sin, op=MUL)
                        nc.vector.tensor_tensor(out=rot[:, :, 32:64],
                                                in0=t3[:], in1=t4[:], op=ADD)
                        rot_tiles[(fp, ns)] = rot

                # ---- transpose q,k into [d, n] layout (SBUF->SBUF DMA) ----
                for fb in range(16):    # 0..7 q blocks, 8..15 k blocks
                    fcol = (fb % 8) * 128
                    for ns in range(4):
                        rot = rot_tiles[(fb // 8, ns)]
                        nc.scalar.dma_start(
                            qkT_g[:, fb, ns * 128:(ns + 1) * 128],
                            rot[:].rearrange("p h d -> p (h d)")[:, fcol:fcol + 128],
                            transpose=True)

                # ---- attention per batch ----
                # sums are split into two 8-head halves so the
                # reciprocal -> DRAM-bounce broadcast chain of half A runs
                # on DVE/DMA while the PE computes heads 8..15, keeping the
                # TensorEngine fed through the softmax normalization.
                for bb in range(2):
                    b_loc = 2 * g + bb
                    pTs = []
                    rbcs = []

                    def half_sums(ha, ps_sum):
                        recip = recip_p.tile([8, 256], f32, tag="recip")
                        nc.vector.reciprocal(recip[:], ps_sum[:])
                        recip_dram = dram_p.tile([8, 256], f32)
                        nc.sync.dma_start(recip_dram[:], recip[:])
                        rbc = recipbc_p.tile([128, 4, 256], bf16, tag="rbc")
                        rd = recip_dram[:].rearrange(
                            "(hp two) i -> two hp i", two=2)
                        for a in range(2):
                            nc.gpsimd.dma_start(
                                rbc[a * 64:(a + 1) * 64, :, :],
                                rd[a][None, :, :].to_broadcast((64, 4, 256)))
                        rbcs.append(rbc)

                    def pv_wave(hps, oT_b):
                        for hp in hps:
                            ps_o = pso_p.tile([128, 256], f32, tag="o")
                            for h2 in range(2):
                                h = hp * 2 + h2
                                for jc in range(2):
                                    nc.tensor.matmul(
                                        ps_o[h2 * 64:h2 * 64 + 64, :],
                                        lhsT=v_g[:, bb * 2 + jc,
                                                 h * 64:(h + 1) * 64],
                                        rhs=pTs[hp * 2 + h2][:, jc, :],
                                        start=(jc == 0), stop=(jc == 1))
                            nc.vector.tensor_tensor(
                                out=oT_b[:, hp, :], in0=ps_o[:],
                                in1=rbcs[hp // 4][:, hp % 4, :], op=MUL)

                    oT_b = oT_p.tile([128, 8, 256], bf16, tag="oT")
                    ps_sum = None
                    for h in range(H):
                        if h % 8 == 0:
                            ps_sum = pssum_p.tile([8, 256], f32, tag="sums")
                        qfb, qrow = h // 2, (h % 2) * 64
                        kfb = 8 + h // 2
                        ps_s = psmm_p.tile([128, 2, 256], f32, tag="mm")
                        for jc in range(2):
                            nc.tensor.matmul(
                                ps_s[:, jc, :],
                                lhsT=qkT_g[qrow:qrow + 64, kfb,
                                           bb * 256 + jc * 128:bb * 256 + jc * 128 + 128],
                                rhs=qkT_g[qrow:qrow + 64, qfb,
                                          bb * 256:bb * 256 + 256],
                                start=True, stop=True)
                        pT = pT_p.tile([128, 2, 256], bf16, tag="pT")
                        nc.scalar.activation(pT[:], ps_s[:], Exp)
                        pTs.append(pT)
                        r = h % 8
                        for jc in range(2):
                            nc.tensor.matmul(
                                ps_sum[:],
                                lhsT=selc[:, 15 - r:23 - r],
                                rhs=pT[:, jc, :],
                                start=(r == 0 and jc == 0),
                                stop=(r == 7 and jc == 1))
                        if h == 7:
                            half_sums(0, ps_sum)
                        if h == 11:
                            pv_wave(range(4), oT_b)
                    half_sums(1, ps_sum)
                    pv_wave(range(4, 8), oT_b)

                    # ---- output projection ----
                    for nt2 in range(2):
                        for oc in range(2):
                            ps_p = psmm_p.tile([128, 2, 256], f32, tag="mm")
                            ps_pv = ps_p[:].rearrange("p a b -> p (a b)")
                            for cc in range(8):
                                nc.tensor.matmul(
                                    ps_pv,
                                    lhsT=oT_b[:, cc, nt2 * 128:(nt2 + 1) * 128],
                                    rhs=pw_sb[:, cc, oc * 512:(oc + 1) * 512],
                                    start=(cc == 0), stop=(cc == 7))
                            y_sb = y_p.tile([128, 512], f32, tag="y")
                            nc.vector.tensor_tensor(
                                out=y_sb[:], in0=ps_pv,
                                in1=pb_bc[:, oc * 512:(oc + 1) * 512], op=ADD)
                            row0 = b_loc * 256 + nt2 * 128
                            nc.sync.dma_start(
                                out_d.ap()[row0:row0 + 128,
                                           oc * 512:(oc + 1) * 512],
                                y_sb[:])

    nc.compile()
    _cache["nc"] = nc
    return nc


def _prep_inputs(x, qkv_w, proj_w, proj_b):
    perm = np.concatenate([np.arange(0, D, 2), np.arange(1, D, 2)])  # evens|odds
    head_perm = (np.arange(H)[:, None] * D + perm[None, :]).reshape(-1)
    wq = qkv_w[:C][head_perm] * np.float32(D ** -0.5)
    wk = qkv_w[C:2 * C][head_perm]
    wv = qkv_w[2 * C:]
    wT = np.ascontiguousarray(
        np.concatenate([wq, wk, wv], 0).T).astype(ml_dtypes.bfloat16)
    pwT = np.ascontiguousarray(proj_w.T).astype(ml_dtypes.bfloat16)
    pb = np.ascontiguousarray(proj_b.reshape(1, C)).astype(np.float32)
    cosH, sinH = _rope_tables()
    cosH = cosH.astype(ml_dtypes.bfloat16)
    sinH = sinH.astype(ml_dtypes.bfloat16)

    in_maps = []
    for c in range(N_CORES):
        xs = x[c * B_LOC:(c + 1) * B_LOC].reshape(NT, C)
        xT = np.ascontiguousarray(xs.T).astype(ml_dtypes.bfloat16)
        in_maps.append({"xT": xT, "wT": wT, "pwT": pwT, "pb": pb,
                        "cosH": cosH, "sinH": sinH})
    return in_maps


def _run(inputs, trace=False, **kw):
    nc = _build()
    in_maps = _prep_inputs(inputs["x"], inputs["qkv_w"],
                           inputs["proj_w"], inputs["proj_b"])
    res = run_bass_kernel_spmd(nc, in_maps, core_ids=list(range(N_CORES)),
                               trace=trace, **kw)
    out = np.concatenate([res.results[c]["out"] for c in range(N_CORES)], 0)
    return out.reshape(B, N, C).astype(np.float32), res


def kernel(x, qkv_w, proj_w, proj_b):
    x = np.asarray(x, dtype=np.float32)
    qkv_w = np.asarray(qkv_w, dtype=np.float32)
    proj_w = np.asarray(proj_w, dtype=np.float32)
    proj_b = np.asarray(proj_b, dtype=np.float32)
    out, _ = _run({"x": x, "qkv_w": qkv_w, "proj_w": proj_w,
                   "proj_b": proj_b})
    return out


# revision 17
# speedup vs baseline: 1.4309x; 1.4309x over previous
"""Trainium2 Bass kernel for nn_Attention_68006512164916.

EVA-style vision attention block: qkv proj -> 2D rope (interleaved pairs)
-> SDPA (16 heads, d=64, seq 256) -> out proj. B=64, N=256, C=1024, fp32 I/O.

Strategy: data-parallel over batch across 8 NeuronCores (8 batches/core,
no collectives). Per core, everything is computed in bf16 on the
TensorEngine with fp32 PSUM accumulation:

  - host: x is transposed/cast to xT [C, B_loc*N] bf16 so the QKV matmul
    needs no on-device transpose of x; qkv_w rows for q/k are permuted
    (per-head d-interleave -> [evens|odds]) so rope becomes half-block
    free-axis ops; q rows pre-scaled by D^-0.5; proj_w pre-transposed.
  - qkv = xT.T @ wT  (option A layout [n, o]) -> PSUM -> bf16 SBUF
  - rope applied as 6 DVE tensor_tensor ops per [128,1024] tile
  - q,k transposed per 128x128 block on the TensorEngine (identity mm)
    into qT/kT [d, n] layout for attention
  - S^T[j,i] = kT.T @ qT (K=64), exp on ScalarE (no max subtraction:
    |scores| <= ~6 for this input distribution), row sums over j via a
    one-hot-selector matmul into a [16, 256] PSUM tile, 1/sum on DVE,
    broadcast via a DRAM bounce, O^T = v.T-free PV matmul, normalize
    fused into the PSUM eviction.
  - y = O^T.T @ pwT + b, streamed out as fp32.
"""

import sys

if "/opt/trn_rl_repo" not in sys.path:
    sys.path.insert(0, "/opt/trn_rl_repo")

import numpy as np
import ml_dtypes

import concourse.bacc as bacc
import concourse.mybir as mybir
import concourse.tile as tile
from concourse.bass_utils import run_bass_kernel_spmd
from concourse.masks import make_identity

f32 = mybir.dt.float32
bf16 = mybir.dt.bfloat16

N_CORES = 8
B, N, C = 64, 256, 1024
H, D = 16, 64
B_LOC = B // N_CORES          # 8 batches per core
NT = B_LOC * N                # 2048 token rows per core
HW = 16
THETA = 10000.0

_cache = {}


def _rope_tables():
    hd = D // 2  # 32
    inv_freq = 1.0 / (THETA ** (np.arange(0, hd, 2, dtype=np.float32) / hd))
    t = np.arange(HW, dtype=np.float32)
    f = np.einsum("i,j->ij", t, inv_freq)          # (16, 16)
    f = np.repeat(f, 2, axis=-1)                   # (16, 32)
    fx = np.broadcast_to(f[:, None, :], (HW, HW, hd))
    fy = np.broadcast_to(f[None, :, :], (HW, HW, hd))
    F = np.concatenate([fx, fy], axis=-1).reshape(N, D)  # (256, 64)
    cosH = np.cos(F[:, 0::2])                      # (256, 32)
    sinH = np.sin(F[:, 0::2])
    return cosH.astype(np.float32), sinH.astype(np.float32)


def _build():
    if "nc" in _cache:
        return _cache["nc"]

    nc = bacc.Bacc("TRN2", target_bir_lowering=False, debug=False,
                   num_devices=N_CORES)

    xT_d = nc.dram_tensor("xT", [C, NT], bf16, kind="ExternalInput")
    w_d = nc.dram_tensor("wT", [C, 3 * C], bf16, kind="ExternalInput")
    pw_d = nc.dram_tensor("pwT", [C, C], bf16, kind="ExternalInput")
    pb_d = nc.dram_tensor("pb", [1, C], f32, kind="ExternalInput")
    cos_d = nc.dram_tensor("cosH", [N, 32], bf16, kind="ExternalInput")
    sin_d = nc.dram_tensor("sinH", [N, 32], bf16, kind="ExternalInput")
    out_d = nc.dram_tensor("out", [NT, C], f32, kind="ExternalOutput")

    Exp = mybir.ActivationFunctionType.Exp
    MUL = mybir.AluOpType.mult
    ADD = mybir.AluOpType.add
    SUB = mybir.AluOpType.subtract

    from contextlib import ExitStack
    with tile.TileContext(nc) as tc:
        with ExitStack() as ctx:
            const = ctx.enter_context(tc.tile_pool(name="const", bufs=1))
            xg_p = ctx.enter_context(tc.tile_pool(name="xg", bufs=2))
            qkraw_p = ctx.enter_context(tc.tile_pool(name="qkraw", bufs=2))
            tmp_p = ctx.enter_context(tc.tile_pool(name="tmp", bufs=4))
            qkrot_p = ctx.enter_context(tc.tile_pool(name="qkrot", bufs=9))
            v_p = ctx.enter_context(tc.tile_pool(name="vg", bufs=2))
            qkT_p = ctx.enter_context(tc.tile_pool(name="qkT", bufs=2))
            pT_p = ctx.enter_context(tc.tile_pool(name="pT", bufs=18))
            oT_p = ctx.enter_context(tc.tile_pool(name="oT", bufs=2))
            recip_p = ctx.enter_context(tc.tile_pool(name="recip", bufs=3))
            recipbc_p = ctx.enter_context(tc.tile_pool(name="recipbc", bufs=3))
            y_p = ctx.enter_context(tc.tile_pool(name="y", bufs=2))
            dram_p = ctx.enter_context(tc.tile_pool(name="dram", bufs=3, space="DRAM"))
            psmm_p = ctx.enter_context(tc.tile_pool(name="psmm", bufs=4, space="PSUM"))
            pso_p = ctx.enter_context(tc.tile_pool(name="pso", bufs=2, space="PSUM"))
            pssum_p = ctx.enter_context(tc.tile_pool(name="pssum", bufs=2, space="PSUM"))

            # ---- constants ----
            # split the 6MB weight load into per-chunk DMAs spread across
            # the per-engine DMA queues so they run in parallel and the
            # first qkv matmuls can start as soon as chunk 0 lands
            dma_engines = [nc.sync, nc.scalar, nc.gpsimd]
            w_sb = const.tile([128, 8, 3 * C], bf16)
            w_r = w_d.ap().rearrange("(co ci) o -> ci co o", ci=128)
            for cc in range(8):
                dma_engines[cc % 3].dma_start(
                    w_sb[:, cc:cc + 1, :], w_r[:, cc:cc + 1, :])
            cos_sb = const.tile([128, 2, 32], bf16)
            nc.sync.dma_start(cos_sb[:], cos_d.ap().rearrange(
                "(nt p) t -> p nt t", p=128))
            sin_sb = const.tile([128, 2, 32], bf16)
            nc.sync.dma_start(sin_sb[:], sin_d.ap().rearrange(
                "(nt p) t -> p nt t", p=128))
            ident = const.tile([128, 128], bf16)
            make_identity(nc, ident)
            # selector for head sums: selc[:, 15] == 1; E_r = selc[:, 15-r:23-r]
            selc = const.tile([128, 31], bf16)
            nc.vector.memset(selc[:], 0.0)
            nc.vector.memset(selc[:, 15:16], 1.0)
            # loads not needed until the first projection (~100us in)
            pw_sb = const.tile([128, 8, C], bf16)
            nc.sync.dma_start(pw_sb[:], pw_d.ap().rearrange(
                "(co ci) o -> ci co o", ci=128))
            pb_bc = const.tile([128, C], f32)
            nc.sync.dma_start(pb_bc[:], pb_d.ap().to_broadcast((128, C)))

            xT_r = xT_d.ap().rearrange("(co ci) n -> ci co n", ci=128)

            for g in range(4):          # group = 2 batches (512 token cols)
                xg = xg_p.tile([128, 8, 512], bf16, tag="xg")
                if g == 0:
                    # split the first x load too: the first matmul only
                    # needs chunk 0, so don't serialize behind 1MB
                    for cc in range(8):
                        dma_engines[(cc + 1) % 3].dma_start(
                            xg[:, cc:cc + 1, :],
                            xT_r[:, cc:cc + 1, g * 512:(g + 1) * 512])
                else:
                    nc.sync.dma_start(xg[:], xT_r[:, :, g * 512:(g + 1) * 512])
                v_g = v_p.tile([128, 4, C], bf16, tag="vg")
                qkT_g = qkT_p.tile([128, 16, 512], bf16, tag="qkT")
                rot_tiles = {}

                # ---- qkv matmuls + rope ----
                # the two 512-wide halves of each 1024-col output share the
                # same stationary lhsT per k-chunk, letting walrus reuse the
                # loaded weights between consecutive matmuls
                for fp in range(3):     # 0: q, 1: k, 2: v (1024 cols each)
                    for ns in range(4):
                        if fp < 2:
                            raw = qkraw_p.tile([128, H, D], bf16, tag="qkraw")
                            rawf = raw[:].rearrange("p h d -> p (h d)")
                        pss = [psmm_p.tile([128, 512], f32, tag="mm",
                                           name=f"ps_{fp}_{ns}_{i}")
                               for i in range(2)]
                        for cc in range(8):
                            for half in range(2):
                                fo = fp * 2 + half
                                nc.tensor.matmul(
                                    pss[half][:],
                                    lhsT=xg[:, cc, ns * 128:(ns + 1) * 128],
                                    rhs=w_sb[:, cc, fo * 512:(fo + 1) * 512],
                                    start=(cc == 0), stop=(cc == 7))
                        for half in range(2):
                            if fp == 2:
                                nc.any.tensor_copy(
                                    out=v_g[:, ns, half * 512:(half + 1) * 512],
                                    in_=pss[half][:])
                            else:
                                nc.any.tensor_copy(
                                    out=rawf[:, half * 512:(half + 1) * 512],
                                    in_=pss[half][:])
                        if fp == 2:
                            continue
                        # rope: evens = raw[:,:,0:32], odds = raw[:,:,32:64]
                        nt = ns % 2
                        cos = cos_sb[:, nt, None, :].to_broadcast((128, H, 32))
                        sin = sin_sb[:, nt, None, :].to_broadcast((128, H, 32))
                        qe = raw[:, :, 0:32]
                        qo = raw[:, :, 32:64]
                        t1 = tmp_p.tile([128, H, 32], bf16, tag="tmp")
                        t2 = tmp_p.tile([128, H, 32], bf16, tag="tmp")
                        nc.vector.tensor_tensor(out=t1[:], in0=qe, in1=cos, op=MUL)
                        nc.vector.tensor_tensor(out=t2[:], in0=qo, in1=sin, op=MUL)
                        rot = qkrot_p.tile([128, H, D], bf16, tag="rot")
                        nc.vector.tensor_tensor(out=rot[:, :, 0:32],
                                                in0=t1[:], in1=t2[:], op=SUB)
                        t3 = tmp_p.tile([128, H, 32], bf16, tag="tmp")
                        t4 = tmp_p.tile([128, H, 32], bf16, tag="tmp")
                        nc.vector.tensor_tensor(out=t3[:], in0=qo, in1=cos, op=MUL)
                        nc.vector.tensor_tensor(out=t4[:], in0=qe, in1=sin, op=MUL)
                        nc.vector.tensor_tensor(out=rot[:, :, 32:64],
                                                in0=t3[:], in1=t4[:], op=ADD)
                        rot_tiles[(fp, ns)] = rot

                # ---- transpose q,k into [d, n] layout (PE identity mm) ----
                for fb in range(16):    # 0..7 q blocks, 8..15 k blocks
                    fcol = (fb % 8) * 128
                    pst = psmm_p.tile([128, 512], bf16, tag="mm",
                                      name=f"pst_{g}_{fb}")
                    for ns in range(4):
                        rot = rot_tiles[(fb // 8, ns)]
                        nc.tensor.transpose(
                            pst[:, ns * 128:(ns + 1) * 128],
                            rot[:].rearrange("p h d -> p (h d)")[:, fcol:fcol + 128],
                            ident)
                    nc.any.tensor_copy(out=qkT_g[:, fb, :], in_=pst[:])

                # ---- attention per batch ----
                # sums are split into two 8-head halves so the
                # reciprocal -> DRAM-bounce broadcast chain of half A runs
                # on DVE/DMA while the PE computes heads 8..15, keeping the
                # TensorEngine fed through the softmax normalization.
                for bb in range(2):
                    b_loc = 2 * g + bb
                    pTs = []
                    rbcs = []

                    def half_sums(ha, ps_sum):
                        recip = recip_p.tile([8, 256], f32, tag="recip")
                        nc.vector.reciprocal(recip[:], ps_sum[:])
                        recip_dram = dram_p.tile([8, 256], f32)
                        nc.sync.dma_start(recip_dram[:], recip[:])
                        rbc = recipbc_p.tile([128, 4, 256], bf16, tag="rbc")
                        rd = recip_dram[:].rearrange(
                            "(hp two) i -> two hp i", two=2)
                        for a in range(2):
                            nc.gpsimd.dma_start(
                                rbc[a * 64:(a + 1) * 64, :, :],
                                rd[a][None, :, :].to_broadcast((64, 4, 256)))
                        rbcs.append(rbc)

                    def pv_wave(hps, oT_b):
                        for hp in hps:
                            ps_o = pso_p.tile([128, 256], f32, tag="o")
                            for h2 in range(2):
                                h = hp * 2 + h2
                                for jc in range(2):
                                    nc.tensor.matmul(
                                        ps_o[h2 * 64:h2 * 64 + 64, :],
                                        lhsT=v_g[:, bb * 2 + jc,
                                                 h * 64:(h + 1) * 64],
                                        rhs=pTs[hp * 2 + h2][:, jc, :],
                                        start=(jc == 0), stop=(jc == 1))
                            nc.vector.tensor_tensor(
                                out=oT_b[:, hp, :], in0=ps_o[:],
                                in1=rbcs[hp // 4][:, hp % 4, :], op=MUL)

                    oT_b = oT_p.tile([128, 8, 256], bf16, tag="oT")
                    ps_sum = None
                    for h in range(H):
                        if h % 8 == 0:
                            ps_sum = pssum_p.tile([8, 256], f32, tag="sums")
                        qfb, qrow = h // 2, (h % 2) * 64
                        kfb = 8 + h // 2
                        ps_s = psmm_p.tile([128, 2, 256], f32, tag="mm")
                        for jc in range(2):
                            nc.tensor.matmul(
                                ps_s[:, jc, :],
                                lhsT=qkT_g[qrow:qrow + 64, kfb,
                                           bb * 256 + jc * 128:bb * 256 + jc * 128 + 128],
                                rhs=qkT_g[qrow:qrow + 64, qfb,
                                          bb * 256:bb * 256 + 256],
                                start=True, stop=True)
                        pT = pT_p.tile([128, 2, 256], bf16, tag="pT")
                        nc.scalar.activation(pT[:], ps_s[:], Exp)
                        pTs.append(pT)
                        r = h % 8
                        for jc in range(2):
                            nc.tensor.matmul(
                                ps_sum[:],
                                lhsT=selc[:, 15 - r:23 - r],
                                rhs=pT[:, jc, :],
                                start=(r == 0 and jc == 0),
                                stop=(r == 7 and jc == 1))
                        if h == 7:
                            half_sums(0, ps_sum)
                        if h == 11:
                            pv_wave(range(4), oT_b)
                    half_sums(1, ps_sum)
                    pv_wave(range(4, 8), oT_b)

                    # ---- output projection ----
                    for nt2 in range(2):
                        for oc in range(2):
                            ps_p = psmm_p.tile([128, 2, 256], f32, tag="mm")
                            ps_pv = ps_p[:].rearrange("p a b -> p (a b)")
                            for cc in range(8):
                                nc.tensor.matmul(
                                    ps_pv,
                                    lhsT=oT_b[:, cc, nt2 * 128:(nt2 + 1) * 128],
                                    rhs=pw_sb[:, cc, oc * 512:(oc + 1) * 512],
                                    start=(cc == 0), stop=(cc == 7))
                            y_sb = y_p.tile([128, 512], f32, tag="y")
                            nc.vector.tensor_tensor(
                                out=y_sb[:], in0=ps_pv,
                                in1=pb_bc[:, oc * 512:(oc + 1) * 512], op=ADD)
                            row0 = b_loc * 256 + nt2 * 128
                            nc.sync.dma_start(
                                out_d.ap()[row0:row0 + 128,
                                           oc * 512:(oc + 1) * 512],
                                y_sb[:])

    nc.compile()
    _cache["nc"] = nc
    return nc


def _prep_inputs(x, qkv_w, proj_w, proj_b):
    perm = np.concatenate([np.arange(0, D, 2), np.arange(1, D, 2)])  # evens|odds
    head_perm = (np.arange(H)[:, None] * D + perm[None, :]).reshape(-1)
    wq = qkv_w[:C][head_perm] * np.float32(D ** -0.5)
    wk = qkv_w[C:2 * C][head_perm]
    wv = qkv_w[2 * C:]
    wT = np.ascontiguousarray(
        np.concatenate([wq, wk, wv], 0).T).astype(ml_dtypes.bfloat16)
    pwT = np.ascontiguousarray(proj_w.T).astype(ml_dtypes.bfloat16)
    pb = np.ascontiguousarray(proj_b.reshape(1, C)).astype(np.float32)
    cosH, sinH = _rope_tables()
    cosH = cosH.astype(ml_dtypes.bfloat16)
    sinH = sinH.astype(ml_dtypes.bfloat16)

    in_maps = []
    for c in range(N_CORES):
        xs = x[c * B_LOC:(c + 1) * B_LOC].reshape(NT, C)
        xT = np.ascontiguousarray(xs.T).astype(ml_dtypes.bfloat16)
        in_maps.append({"xT": xT, "wT": wT, "pwT": pwT, "pb": pb,
                        "cosH": cosH, "sinH": sinH})
    return in_maps


def _run(inputs, trace=False, **kw):
    nc = _build()
    in_maps = _prep_inputs(inputs["x"], inputs["qkv_w"],
                           inputs["proj_w"], inputs["proj_b"])
    res = run_bass_kernel_spmd(nc, in_maps, core_ids=list(range(N_CORES)),
                               trace=trace, **kw)
    out = np.concatenate([res.results[c]["out"] for c in range(N_CORES)], 0)
    return out.reshape(B, N, C).astype(np.float32), res


def kernel(x, qkv_w, proj_w, proj_b):
    x = np.asarray(x, dtype=np.float32)
    qkv_w = np.asarray(qkv_w, dtype=np.float32)
    proj_w = np.asarray(proj_w, dtype=np.float32)
    proj_b = np.asarray(proj_b, dtype=np.float32)
    out, _ = _run({"x": x, "qkv_w": qkv_w, "proj_w": proj_w,
                   "proj_b": proj_b})
    return out


# revision 19
# speedup vs baseline: 1.5303x; 1.0694x over previous
"""Trainium2 Bass kernel for nn_Attention_68006512164916.

EVA-style vision attention block: qkv proj -> 2D rope (interleaved pairs)
-> SDPA (16 heads, d=64, seq 256) -> out proj. B=64, N=256, C=1024, fp32 I/O.

Strategy: data-parallel over batch across 8 NeuronCores (8 batches/core,
no collectives). Per core, everything is computed in bf16 on the
TensorEngine with fp32 PSUM accumulation:

  - host: x is transposed/cast to xT [C, B_loc*N] bf16 so the QKV matmul
    needs no on-device transpose of x; qkv_w rows for q/k are permuted
    (per-head d-interleave -> [evens|odds]) so rope becomes half-block
    free-axis ops; q rows pre-scaled by D^-0.5; proj_w pre-transposed.
  - qkv = xT.T @ wT  (option A layout [n, o]) -> PSUM -> bf16 SBUF
  - rope applied as 6 DVE tensor_tensor ops per [128,1024] tile
  - q,k transposed per 128x128 block on the TensorEngine (identity mm)
    into qT/kT [d, n] layout for attention
  - S^T[j,i] = kT.T @ qT (K=64), exp on ScalarE (no max subtraction:
    |scores| <= ~6 for this input distribution), row sums over j via a
    one-hot-selector matmul into a [16, 256] PSUM tile, 1/sum on DVE,
    broadcast via a DRAM bounce, O^T = v.T-free PV matmul, normalize
    fused into the PSUM eviction.
  - y = O^T.T @ pwT + b, streamed out as fp32.
"""

import sys

if "/opt/trn_rl_repo" not in sys.path:
    sys.path.insert(0, "/opt/trn_rl_repo")

import numpy as np
import ml_dtypes

import concourse.bacc as bacc
import concourse.mybir as mybir
import concourse.tile as tile
from concourse.bass_utils import run_bass_kernel_spmd
from concourse.masks import make_identity

f32 = mybir.dt.float32
bf16 = mybir.dt.bfloat16

N_CORES = 8
B, N, C = 64, 256, 1024
H, D = 16, 64
B_LOC = B // N_CORES          # 8 batches per core
NT = B_LOC * N                # 2048 token rows per core
HW = 16
THETA = 10000.0

_cache = {}


def _rope_tables():
    hd = D // 2  # 32
    inv_freq = 1.0 / (THETA ** (np.arange(0, hd, 2, dtype=np.float32) / hd))
    t = np.arange(HW, dtype=np.float32)
    f = np.einsum("i,j->ij", t, inv_freq)          # (16, 16)
    f = np.repeat(f, 2, axis=-1)                   # (16, 32)
    fx = np.broadcast_to(f[:, None, :], (HW, HW, hd))
    fy = np.broadcast_to(f[None, :, :], (HW, HW, hd))
    F = np.concatenate([fx, fy], axis=-1).reshape(N, D)  # (256, 64)
    cosH = np.cos(F[:, 0::2])                      # (256, 32)
    sinH = np.sin(F[:, 0::2])
    return cosH.astype(np.float32), sinH.astype(np.float32)


def _build():
    if "nc" in _cache:
        return _cache["nc"]

    nc = bacc.Bacc("TRN2", target_bir_lowering=False, debug=False,
                   num_devices=N_CORES)

    xT_d = nc.dram_tensor("xT", [C, NT], bf16, kind="ExternalInput")
    w_d = nc.dram_tensor("wT", [C, 3 * C], bf16, kind="ExternalInput")
    pw_d = nc.dram_tensor("pwT", [C, C], bf16, kind="ExternalInput")
    pb_d = nc.dram_tensor("pb", [1, C], f32, kind="ExternalInput")
    cos_d = nc.dram_tensor("cosH", [N, 32], bf16, kind="ExternalInput")
    sin_d = nc.dram_tensor("sinH", [N, 32], bf16, kind="ExternalInput")
    out_d = nc.dram_tensor("out", [NT, C], f32, kind="ExternalOutput")

    Exp = mybir.ActivationFunctionType.Exp
    MUL = mybir.AluOpType.mult
    ADD = mybir.AluOpType.add
    SUB = mybir.AluOpType.subtract

    from contextlib import ExitStack
    with tile.TileContext(nc) as tc:
        with ExitStack() as ctx:
            const = ctx.enter_context(tc.tile_pool(name="const", bufs=1))
            xg_p = ctx.enter_context(tc.tile_pool(name="xg", bufs=2))
            qkraw_p = ctx.enter_context(tc.tile_pool(name="qkraw", bufs=2))
            tmp_p = ctx.enter_context(tc.tile_pool(name="tmp", bufs=4))
            qkrot_p = ctx.enter_context(tc.tile_pool(name="qkrot", bufs=9))
            v_p = ctx.enter_context(tc.tile_pool(name="vg", bufs=2))
            qkT_p = ctx.enter_context(tc.tile_pool(name="qkT", bufs=2))
            pT_p = ctx.enter_context(tc.tile_pool(name="pT", bufs=18))
            oT_p = ctx.enter_context(tc.tile_pool(name="oT", bufs=2))
            recip_p = ctx.enter_context(tc.tile_pool(name="recip", bufs=3))
            recipbc_p = ctx.enter_context(tc.tile_pool(name="recipbc", bufs=3))
            y_p = ctx.enter_context(tc.tile_pool(name="y", bufs=2))
            dram_p = ctx.enter_context(tc.tile_pool(name="dram", bufs=3, space="DRAM"))
            psmm_p = ctx.enter_context(tc.tile_pool(name="psmm", bufs=4, space="PSUM"))
            pso_p = ctx.enter_context(tc.tile_pool(name="pso", bufs=2, space="PSUM"))
            pssum_p = ctx.enter_context(tc.tile_pool(name="pssum", bufs=2, space="PSUM"))

            # ---- constants ----
            # split the 6MB weight load into per-chunk DMAs spread across
            # the per-engine DMA queues so they run in parallel and the
            # first qkv matmuls can start as soon as chunk 0 lands
            dma_engines = [nc.sync, nc.scalar, nc.gpsimd]
            w_sb = const.tile([128, 8, 3 * C], bf16)
            w_r = w_d.ap().rearrange("(co ci) o -> ci co o", ci=128)
            cos_sb = const.tile([128, 2, 32], bf16)
            nc.sync.dma_start(cos_sb[:], cos_d.ap().rearrange(
                "(nt p) t -> p nt t", p=128))
            sin_sb = const.tile([128, 2, 32], bf16)
            nc.sync.dma_start(sin_sb[:], sin_d.ap().rearrange(
                "(nt p) t -> p nt t", p=128))
            ident = const.tile([128, 128], bf16)
            make_identity(nc, ident)
            # selector for head sums: selc[:, 15] == 1; E_r = selc[:, 15-r:23-r]
            selc = const.tile([128, 31], bf16)
            nc.vector.memset(selc[:], 0.0)
            nc.vector.memset(selc[:, 15:16], 1.0)
            # loads not needed until the first projection (~100us in)
            pw_sb = const.tile([128, 8, C], bf16)
            nc.sync.dma_start(pw_sb[:], pw_d.ap().rearrange(
                "(co ci) o -> ci co o", ci=128))
            pb_bc = const.tile([128, C], f32)
            nc.sync.dma_start(pb_bc[:], pb_d.ap().to_broadcast((128, C)))

            xT_r = xT_d.ap().rearrange("(co ci) n -> ci co n", ci=128)
            pending = [None]

            for g in range(4):          # group = 2 batches (512 token cols)
                xg = xg_p.tile([128, 8, 512], bf16, tag="xg")
                if g == 0:
                    # interleaved per-chunk loads: the first matmul needs
                    # only (xg chunk 0, w chunk 0), so land those first,
                    # spread round-robin across the three DMA-capable
                    # engine queues
                    for cc in range(8):
                        dma_engines[(2 * cc) % 3].dma_start(
                            xg[:, cc:cc + 1, :],
                            xT_r[:, cc:cc + 1, g * 512:(g + 1) * 512])
                        dma_engines[(2 * cc + 1) % 3].dma_start(
                            w_sb[:, cc:cc + 1, :], w_r[:, cc:cc + 1, :])
                else:
                    nc.sync.dma_start(xg[:], xT_r[:, :, g * 512:(g + 1) * 512])
                v_g = v_p.tile([128, 4, C], bf16, tag="vg")
                qkT_g = qkT_p.tile([128, 16, 512], bf16, tag="qkT")
                rot_tiles = {}

                # ---- qkv matmuls + rope ----
                # the two 512-wide halves of each 1024-col output share the
                # same stationary lhsT per k-chunk, letting walrus reuse the
                # loaded weights between consecutive matmuls
                for fp in range(3):     # 0: q, 1: k, 2: v (1024 cols each)
                    for ns in range(4):
                        if fp < 2:
                            raw = qkraw_p.tile([128, H, D], bf16, tag="qkraw")
                            rawf = raw[:].rearrange("p h d -> p (h d)")
                        pss = [psmm_p.tile([128, 512], f32, tag="mm",
                                           name=f"ps_{fp}_{ns}_{i}")
                               for i in range(2)]
                        for cc in range(8):
                            for half in range(2):
                                fo = fp * 2 + half
                                nc.tensor.matmul(
                                    pss[half][:],
                                    lhsT=xg[:, cc, ns * 128:(ns + 1) * 128],
                                    rhs=w_sb[:, cc, fo * 512:(fo + 1) * 512],
                                    start=(cc == 0), stop=(cc == 7))
                        for half in range(2):
                            if fp == 2:
                                nc.any.tensor_copy(
                                    out=v_g[:, ns, half * 512:(half + 1) * 512],
                                    in_=pss[half][:])
                            else:
                                nc.any.tensor_copy(
                                    out=rawf[:, half * 512:(half + 1) * 512],
                                    in_=pss[half][:])
                        if fp == 2:
                            continue
                        # rope: evens = raw[:,:,0:32], odds = raw[:,:,32:64]
                        nt = ns % 2
                        cos = cos_sb[:, nt, None, :].to_broadcast((128, H, 32))
                        sin = sin_sb[:, nt, None, :].to_broadcast((128, H, 32))
                        qe = raw[:, :, 0:32]
                        qo = raw[:, :, 32:64]
                        t1 = tmp_p.tile([128, H, 32], bf16, tag="tmp")
                        t2 = tmp_p.tile([128, H, 32], bf16, tag="tmp")
                        nc.vector.tensor_tensor(out=t1[:], in0=qe, in1=cos, op=MUL)
                        nc.vector.tensor_tensor(out=t2[:], in0=qo, in1=sin, op=MUL)
                        rot = qkrot_p.tile([128, H, D], bf16, tag="rot")
                        nc.vector.tensor_tensor(out=rot[:, :, 0:32],
                                                in0=t1[:], in1=t2[:], op=SUB)
                        t3 = tmp_p.tile([128, H, 32], bf16, tag="tmp")
                        t4 = tmp_p.tile([128, H, 32], bf16, tag="tmp")
                        nc.vector.tensor_tensor(out=t3[:], in0=qo, in1=cos, op=MUL)
                        nc.vector.tensor_tensor(out=t4[:], in0=qe, in1=sin, op=MUL)
                        nc.vector.tensor_tensor(out=rot[:, :, 32:64],
                                                in0=t3[:], in1=t4[:], op=ADD)
                        rot_tiles[(fp, ns)] = rot

                # ---- transpose q,k into [d, n] layout (PE identity mm) ----
                for fb in range(16):    # 0..7 q blocks, 8..15 k blocks
                    fcol = (fb % 8) * 128
                    pst = psmm_p.tile([128, 512], bf16, tag="mm",
                                      name=f"pst_{g}_{fb}")
                    for ns in range(4):
                        rot = rot_tiles[(fb // 8, ns)]
                        nc.tensor.transpose(
                            pst[:, ns * 128:(ns + 1) * 128],
                            rot[:].rearrange("p h d -> p (h d)")[:, fcol:fcol + 128],
                            ident)
                    nc.any.tensor_copy(out=qkT_g[:, fb, :], in_=pst[:])

                # ---- attention per batch ----
                # (pending_proj from the previous batch is emitted inside
                # this batch's head loop so its PSUM-accumulation tail,
                # which waits on the wave-2 O evictions, overlaps scores)
                # sums are split into two 8-head halves so the
                # reciprocal -> DRAM-bounce broadcast chain of half A runs
                # on DVE/DMA while the PE computes heads 8..15, keeping the
                # TensorEngine fed through the softmax normalization.
                for bb in range(2):
                    b_loc = 2 * g + bb
                    pTs = []
                    rbcs = []

                    def half_sums(ha, ps_sum):
                        recip = recip_p.tile([8, 256], f32, tag="recip")
                        nc.vector.reciprocal(recip[:], ps_sum[:])
                        recip_dram = dram_p.tile([8, 256], f32)
                        nc.sync.dma_start(recip_dram[:], recip[:])
                        rbc = recipbc_p.tile([128, 4, 256], bf16, tag="rbc")
                        rd = recip_dram[:].rearrange(
                            "(hp two) i -> two hp i", two=2)
                        for a in range(2):
                            nc.gpsimd.dma_start(
                                rbc[a * 64:(a + 1) * 64, :, :],
                                rd[a][None, :, :].to_broadcast((64, 4, 256)))
                        rbcs.append(rbc)

                    def pv_wave(hps, oT_b):
                        for hp in hps:
                            ps_o = pso_p.tile([128, 256], f32, tag="o")
                            for h2 in range(2):
                                h = hp * 2 + h2
                                for jc in range(2):
                                    nc.tensor.matmul(
                                        ps_o[h2 * 64:h2 * 64 + 64, :],
                                        lhsT=v_g[:, bb * 2 + jc,
                                                 h * 64:(h + 1) * 64],
                                        rhs=pTs[hp * 2 + h2][:, jc, :],
                                        start=(jc == 0), stop=(jc == 1))
                            nc.vector.tensor_tensor(
                                out=oT_b[:, hp, :], in0=ps_o[:],
                                in1=rbcs[hp // 4][:, hp % 4, :], op=MUL)

                    oT_b = oT_p.tile([128, 8, 256], bf16, tag="oT")
                    ps_sum = None
                    for h in range(H):
                        if h % 8 == 0:
                            ps_sum = pssum_p.tile([8, 256], f32, tag="sums")
                        qfb, qrow = h // 2, (h % 2) * 64
                        kfb = 8 + h // 2
                        ps_s = psmm_p.tile([128, 2, 256], f32, tag="mm")
                        for jc in range(2):
                            nc.tensor.matmul(
                                ps_s[:, jc, :],
                                lhsT=qkT_g[qrow:qrow + 64, kfb,
                                           bb * 256 + jc * 128:bb * 256 + jc * 128 + 128],
                                rhs=qkT_g[qrow:qrow + 64, qfb,
                                          bb * 256:bb * 256 + 256],
                                start=True, stop=True)
                        pT = pT_p.tile([128, 2, 256], bf16, tag="pT")
                        nc.scalar.activation(pT[:], ps_s[:], Exp)
                        pTs.append(pT)
                        r = h % 8
                        for jc in range(2):
                            nc.tensor.matmul(
                                ps_sum[:],
                                lhsT=selc[:, 15 - r:23 - r],
                                rhs=pT[:, jc, :],
                                start=(r == 0 and jc == 0),
                                stop=(r == 7 and jc == 1))
                        if h == 1 and pending[0] is not None:
                            pending[0]()
                            pending[0] = None
                        if h == 7:
                            half_sums(0, ps_sum)
                        if h == 11:
                            pv_wave(range(4), oT_b)
                    half_sums(1, ps_sum)
                    pv_wave(range(4, 8), oT_b)

                    # ---- output projection (deferred) ----
                    def make_proj(b_loc, oT_b):
                      def do_proj():
                        for nt2 in range(2):
                          for oc in range(2):
                            ps_p = psmm_p.tile([128, 2, 256], f32, tag="mm")
                            ps_pv = ps_p[:].rearrange("p a b -> p (a b)")
                            for cc in range(8):
                                nc.tensor.matmul(
                                    ps_pv,
                                    lhsT=oT_b[:, cc, nt2 * 128:(nt2 + 1) * 128],
                                    rhs=pw_sb[:, cc, oc * 512:(oc + 1) * 512],
                                    start=(cc == 0), stop=(cc == 7))
                            y_sb = y_p.tile([128, 512], f32, tag="y")
                            nc.vector.tensor_tensor(
                                out=y_sb[:], in0=ps_pv,
                                in1=pb_bc[:, oc * 512:(oc + 1) * 512], op=ADD)
                            row0 = b_loc * 256 + nt2 * 128
                            nc.sync.dma_start(
                                out_d.ap()[row0:row0 + 128,
                                           oc * 512:(oc + 1) * 512],
                                y_sb[:])
                      return do_proj
                    pending[0] = make_proj(b_loc, oT_b)

            if pending[0] is not None:
                pending[0]()
                pending[0] = None

    nc.compile()
    _cache["nc"] = nc
    return nc


def _prep_inputs(x, qkv_w, proj_w, proj_b):
    perm = np.concatenate([np.arange(0, D, 2), np.arange(1, D, 2)])  # evens|odds
    head_perm = (np.arange(H)[:, None] * D + perm[None, :]).reshape(-1)
    wq = qkv_w[:C][head_perm] * np.float32(D ** -0.5)
    wk = qkv_w[C:2 * C][head_perm]
    wv = qkv_w[2 * C:]
    wT = np.ascontiguousarray(
        np.concatenate([wq, wk, wv], 0).T).astype(ml_dtypes.bfloat16)
    pwT = np.ascontiguousarray(proj_w.T).astype(ml_dtypes.bfloat16)
    pb = np.ascontiguousarray(proj_b.reshape(1, C)).astype(np.float32)
    cosH, sinH = _rope_tables()
    cosH = cosH.astype(ml_dtypes.bfloat16)
    sinH = sinH.astype(ml_dtypes.bfloat16)

    in_maps = []
    for c in range(N_CORES):
        xs = x[c * B_LOC:(c + 1) * B_LOC].reshape(NT, C)
        xT = np.ascontiguousarray(xs.T).astype(ml_dtypes.bfloat16)
        in_maps.append({"xT": xT, "wT": wT, "pwT": pwT, "pb": pb,
                        "cosH": cosH, "sinH": sinH})
    return in_maps


def _run(inputs, trace=False, **kw):
    nc = _build()
    in_maps = _prep_inputs(inputs["x"], inputs["qkv_w"],
                           inputs["proj_w"], inputs["proj_b"])
    res = run_bass_kernel_spmd(nc, in_maps, core_ids=list(range(N_CORES)),
                               trace=trace, **kw)
    out = np.concatenate([res.results[c]["out"] for c in range(N_CORES)], 0)
    return out.reshape(B, N, C).astype(np.float32), res


def kernel(x, qkv_w, proj_w, proj_b):
    x = np.asarray(x, dtype=np.float32)
    qkv_w = np.asarray(qkv_w, dtype=np.float32)
    proj_w = np.asarray(proj_w, dtype=np.float32)
    proj_b = np.asarray(proj_b, dtype=np.float32)
    out, _ = _run({"x": x, "qkv_w": qkv_w, "proj_w": proj_w,
                   "proj_b": proj_b})
    return out
